# revision 69
# baseline (speedup 1.0000x reference)
"""Trainium2 Bass kernel for pair-masked causal self-attention.

Problem: B=4, T=2048, C=1024, H=16 heads (hd=64), GPT-style CausalSelfAttention
with a modified causal mask (odd query i cannot attend to i-1).

Sharding: 8 cores = 4 batches x 2 head-groups (8 heads each). No collectives:
each core computes a partial c_proj output (its 512 channels of y), partials
are summed pairwise on the host.

Cost-model-aware design (TimelineSim charges a matmul only for its OUTPUT
free-dim size; contract dim, partition count, and LdWeights are free; each
DMA instruction pays ~1.2us of serialized SP/HWDGE issue overhead):
- q,k computed TRANSPOSED [c_out, t]; scores land as ST[k_pos, q_pos].
- AV is emitted TRANSPOSED: es[k, q] is the stationary operand, the moving
  operand is [v | ones] of shape [128k, 65], so each 128-key accumulation
  step costs only 65 columns (vs 512 in the natural orientation) and yields
  y[q, c] plus the softmax denominator in column 64. AV emission lags its
  score/exp unit by DEFER units (es ring sized to match) so the PE never
  heads-of-line-blocks on an exp. Normalization is a per-partition
  reciprocal + broadcast multiply (q on partitions), then a PE transpose
  (+copy) restores yT[c, q] for the projection; the PE half of the norm is
  deferred ~3 units so the DVE chain hides behind other matmuls, and the
  last pair's is carried into the next window.
- v bias is folded into v itself (softmax weights sum to 1, so
  y = sum(w*(v+bv)) = y + bv exactly); bv arrives host-replicated and is
  added in the existing PSUM->SBUF copy. No per-tile bias matmul.
- Causal + pair masking is ONE static 128x128 mask applied to the diagonal
  sub-block; fully-masked tiles are never computed, diagonal tiles are
  column-trimmed on both the score and AV sides.
- Attention for block n runs INSIDE window n (it only needs qT(n), emitted
  first, plus kT/v from blocks <= n), so the ACT-heavy exp stream overlaps
  the same block's qkv matmuls; proj(n-1) fills the window head.
- Inputs arrive in 11 consolidated DMAs (weights/x as single multi-tile
  transfers, constants packed into two tensors); outputs leave in ~9
  wide transfers. Window 0 runs qk kc-major across all 8 head-blocks
  (8 borrowed PSUM regions) so each weight-chunk arrival unlocks a full
  burst of matmuls.
- Block-3 projection is split: the pair-0..2 contraction runs as pair-3
  filler in window 3 and is staged to SBUF; the drain adds the pair-3
  term plus the staged partial back via an identity matmul, so the drain
  is ~3.5us of PE instead of ~7.
"""

from collections import deque

import numpy as np
import ml_dtypes

import concourse.bass as bass
import concourse.bacc as bacc
import concourse.tile as tile
from concourse import mybir
from concourse.bass_utils import run_bass_kernel_spmd

B, T, C, H = 4, 2048, 1024, 16
HD = C // H          # 64
G = 8                # cores
HPC = 8              # heads per core
PAIRS = HPC // 2     # head pairs per core
NT = T // 512        # 4 q/t column blocks of 512
KT = T // 128        # 16 k row tiles of 128
CT = C // 128        # 8 c_in tiles

DT_NAME = "bfloat16"
DEFER = 6            # AV lags the score/exp unit by this many units

_cache = {}


def _dt(dt_name):
    return getattr(mybir.dt, dt_name)


def _np_dt(dt_name):
    return np.float32 if dt_name == "float32r" else ml_dtypes.bfloat16


def build_nc(dt_name=DT_NAME):
    """Build (and cache) the per-core Bass program."""
    if dt_name in _cache:
        return _cache[dt_name]

    DT = _dt(dt_name)
    F32 = mybir.dt.float32
    nc = bacc.Bacc()

    xT_d = nc.declare_dram_parameter("xT", [C, T], DT, isOutput=False)
    wqk_d = nc.declare_dram_parameter("wqkT", [C, 1024], DT, isOutput=False)
    wv_d = nc.declare_dram_parameter("wvT", [C, 512], DT, isOutput=False)
    wp_d = nc.declare_dram_parameter("wprojT", [512, C], DT, isOutput=False)
    cstf_d = nc.declare_dram_parameter("cstf", [128, 136], F32, isOutput=False)
    cstb_d = nc.declare_dram_parameter("cstb", [128, 896], DT, isOutput=False)
    out_d = nc.declare_dram_parameter("out", [C, T], DT, isOutput=True)

    EXP = mybir.ActivationFunctionType.Exp
    SCALE = 1.0 / float(np.sqrt(HD))

    with tile.TileContext(nc) as tc:
        with (
            tc.tile_pool(name="persist", bufs=1) as persist,
            tc.tile_pool(name="xw", bufs=1) as xw,
            tc.tile_pool(name="attnc", bufs=1) as attnc,
            tc.tile_pool(name="es_p", bufs=16) as es_p,
            tc.tile_pool(name="rec_p", bufs=3) as rec_p,
            tc.tile_pool(name="ystg_p", bufs=3) as ystg_p,
            tc.tile_pool(name="stage_p", bufs=3) as stage_p,
            tc.tile_pool(name="mm_ps", bufs=2, space="PSUM") as mm_ps,
            tc.tile_pool(name="st_ps", bufs=2, space="PSUM") as st_ps,
            tc.tile_pool(name="acc_ps", bufs=2, space="PSUM") as acc_ps,
        ):
            # ---- persistent tensors ----
            qkT = [persist.tile([128, T], DT, tag=f"qkT{m}", name=f"qkT{m}") for m in range(8)]
            v_r = [persist.tile([128, PAIRS, 2, 65], DT, tag=f"vr{t}", name=f"vr{t}") for t in range(KT)]
            yT = [persist.tile([128, T], DT, tag=f"yT{p}", name=f"yT{p}") for p in range(PAIRS)]
            prt = [persist.tile([128, 512], DT, tag=f"prt{o}", name=f"prt{o}") for o in range(8)]

            # ---- input tiles (consolidated) ----
            xT2 = xw.tile([128, CT, T], DT, tag="xT2", name="xT2")
            wqk2 = xw.tile([128, CT, 1024], DT, tag="wqk2", name="wqk2")
            wv2 = xw.tile([128, CT, 512], DT, tag="wv2", name="wv2")
            wp2 = attnc.tile([128, 4, C], DT, tag="wp2", name="wp2")
            cstf = attnc.tile([128, 136], F32, tag="cstf")
            cstb = attnc.tile([128, 896], DT, tag="cstb")
            ones_r = attnc.tile([1, 128], DT, tag="ones")

            bqk = cstf[:, 0:8]
            ident = cstf[:, 8:136]
            msk = cstb[:, 0:256].rearrange("p (h q) -> p h q", h=2)
            bvr = cstb[:, 256:768].rearrange("p (pr two d) -> p pr two d",
                                             pr=PAIRS, two=2)
            identb = cstb[:, 768:896]

            def xTt(kc):
                return xT2[:, kc, :]

            def wqkt(kc):
                return wqk2[:, kc, :]

            nc.vector.memset(ones_r, 1.0)
            for t in range(KT):
                nc.vector.memset(v_r[t][:, :, :, 64:65], 1.0)
            warm = attnc.tile([1, 1], DT, tag="warm")
            nc.scalar.activation(warm, ones_r[:, 0:1], EXP, scale=1.0)
            # PE warm-up: dummy matmuls during the input-DMA wait keep the
            # HAM activity window busy so real matmuls start at full clock
            wps = mm_ps.tile([128, 128], F32, tag="mm", name="warm_ps")

            def warmup(k):
                for _ in range(k):
                    nc.tensor.matmul(wps, ones_r, ones_r, start=True, stop=True,
                                     skip_group_check=True)

            warmup(47)

            # ---- input DMAs ----
            xr = xT_d.rearrange("(kc p) t -> p kc t", p=128)
            wr = wqk_d.rearrange("(kc p) w -> p kc w", p=128)
            nc.sync.dma_start(xT2[:, 0:4, 0:512], xr[:, 0:4, 0:512])
            nc.sync.dma_start(wqk2[:, 0:2, :], wr[:, 0:2, :])
            nc.sync.dma_start(wqk2[:, 2:4, :], wr[:, 2:4, :])
            nc.sync.dma_start(xT2[:, 4:8, 0:512], xr[:, 4:8, 0:512])
            nc.sync.dma_start(wqk2[:, 4:6, :], wr[:, 4:6, :])
            nc.sync.dma_start(wqk2[:, 6:8, :], wr[:, 6:8, :])
            nc.sync.dma_start(wv2, wv_d.rearrange("(kc p) w -> p kc w", p=128))
            nc.sync.dma_start(cstf, cstf_d[:])
            nc.sync.dma_start(cstb, cstb_d[:])
            nc.sync.dma_start(xT2[:, :, 512:T], xr[:, :, 512:T])
            nc.sync.dma_start(wp2, wp_d.rearrange("(kc p) w -> p kc w", p=128))

            # ---- psum region helpers ----
            _uid = [0]

            def uname(pfx):
                _uid[0] += 1
                return f"{pfx}_{_uid[0]}"

            def mm_region():
                return mm_ps.tile([128, 512], F32, tag="mm", name=uname("mm"))

            def st_region():
                return st_ps.tile([128, 2, 512], F32, tag="st", name=uname("stb"))

            def acc_region():
                return acc_ps.tile([128, 4, 128], F32, tag="acc", name=uname("accb"))

            # ---- qkv building blocks ----
            def qk_group(n, m):
                """One q/k head-block m of t-block n: 8 accumulating matmuls
                + bias add into qkT[m]. Emits inline."""
                tsl = bass.ts(n, 512)
                ps = mm_region()
                for kc in range(CT):
                    nc.tensor.matmul(ps, wqkt(kc)[:, 128 * m:128 * m + 128],
                                     xTt(kc)[:, tsl],
                                     start=(kc == 0), stop=(kc == CT - 1))
                nc.vector.tensor_scalar_add(qkT[m][:, tsl], ps, bqk[:, m:m + 1])

            def v_group(tt):
                def g():
                    ps = mm_region()
                    for kc in range(CT):
                        nc.tensor.matmul(ps, xTt(kc)[:, 128 * tt:128 * tt + 128],
                                         wv2[:, kc, :],
                                         start=(kc == 0), stop=(kc == CT - 1))
                    psv = ps.rearrange("p (pr two d) -> p pr two d", pr=PAIRS, two=2)
                    nc.vector.tensor_add(v_r[tt][:, :, :, 0:64], psv, bvr)
                return g

            # ---- attention (transposed AV) ----
            def attn_units(j, lo=0.0, hi=1.0):
                """Returns ([(pos, fn)], last_normB_fn). Pair p's units sit in
                [lo,hi] mapped by fraction; the final pair's PE-side norm is
                returned separately so the caller can weave it into the next
                window (there is no same-window PE work left to hide it)."""
                kk_hi = 4 * j + 4
                qsl0 = 512 * j
                units = []

                seq = list(range(kk_hi))
                k_first = 0

                def emit_av(p, state, kk, es, d):
                    # ONE start per acc bank: start=True marks the whole 2KB
                    # zero region pending, so a second start would re-flag
                    # sibling s-blocks and their next write would overwrite
                    # instead of accumulate. s>0 first-writes rely on the
                    # pending-zero bytes to behave as an implicit start.
                    s_lo = d if d > 0 else 0
                    for h in (0, 1):
                        acc = state["A"] if h == 0 else state["B"]
                        for s in range(s_lo, 4):
                            nc.tensor.matmul(
                                acc[:, s, 0:65], es[:, h, 128 * s:128 * s + 128],
                                v_r[kk][:, p, h, :],
                                start=(kk == 0 and s == 0),
                                stop=(kk == 4 * j + s),
                                skip_group_check=True)

                def mk_unit(p, kk, state, pend):
                    def u():
                        if kk == k_first:
                            state["A"] = acc_region()
                            state["B"] = acc_region()
                        d = kk - 4 * j
                        q0 = 128 * d if d >= 0 else 0
                        st = st_region()
                        kT_t = qkT[4 + p]
                        qT_t = qkT[p]
                        ksl = bass.ts(kk, 128)
                        qsl = bass.ds(qsl0 + q0, 512 - q0)
                        nc.tensor.matmul(st[:, 0, q0:512],
                                         kT_t[0:64, ksl], qT_t[0:64, qsl],
                                         start=True, stop=True)
                        nc.tensor.matmul(st[:, 1, q0:512],
                                         kT_t[64:128, ksl], qT_t[64:128, qsl],
                                         start=True, stop=True)
                        es = es_p.tile([128, 2, 512], DT, tag="es", name=uname("es"))
                        nc.scalar.activation(es[:, :, q0:512], st[:, :, q0:512],
                                             EXP, scale=SCALE)
                        if d >= 0:
                            nc.vector.tensor_mul(es[:, :, q0:q0 + 128],
                                                 es[:, :, q0:q0 + 128], msk)
                        pend.append((kk, es, d))
                        while len(pend) > DEFER:
                            emit_av(p, state, *pend.popleft())
                    return u

                def mk_normA(p, state, pend):
                    def u():
                        while pend:
                            emit_av(p, state, *pend.popleft())
                        accA, accB = state["A"], state["B"]
                        rec = rec_p.tile([128, 2, 4], F32, tag="rec", name=uname("rec"))
                        nc.vector.reciprocal_approx_fast(out=rec[:, 0, :],
                                                         in_=accA[:, :, 64])
                        nc.vector.reciprocal_approx_fast(out=rec[:, 1, :],
                                                         in_=accB[:, :, 64])
                        ystg = ystg_p.tile([128, 4, 128], F32, tag="ystg",
                                           name=uname("ystg"))
                        nc.vector.tensor_mul(ystg[:, :, 0:64], accA[:, :, 0:64],
                                             rec[:, 0, :].broadcast_to([128, 4, 64]))
                        nc.vector.tensor_mul(ystg[:, :, 64:128], accB[:, :, 0:64],
                                             rec[:, 1, :].broadcast_to([128, 4, 64]))
                        state["ystg"] = ystg
                    return u

                def mk_normB(p, state, on_act=False):
                    def u():
                        ystg = state["ystg"]
                        tp = mm_region().rearrange("p (a b) -> p a b", a=4)
                        for s in range(4):
                            nc.tensor.matmul(tp[:, s, :], ystg[:, s, :], ident,
                                             is_transpose=True, start=(s == 0),
                                             stop=(s == 3), skip_group_check=True)
                        if on_act:
                            # final carry only: ACT is idle at drain time and
                            # the yT3 copy gates every finish group
                            nc.scalar.activation(
                                yT[p][:, qsl0:qsl0 + 512],
                                tp.rearrange("p a b -> p (a b)"),
                                mybir.ActivationFunctionType.Copy)
                        else:
                            nc.vector.tensor_copy(yT[p][:, qsl0:qsl0 + 512],
                                                  tp.rearrange("p a b -> p (a b)"))
                    return u

                last_normB = None
                for p in range(PAIRS):
                    state = {}
                    pend = deque()
                    for i, kk in enumerate(seq):
                        frac = (p + (i + 0.5) / (kk_hi + 1)) / PAIRS
                        units.append((lo + (hi - lo) * frac, mk_unit(p, kk, state, pend)))
                    fa = (p + (kk_hi + 0.5) / (kk_hi + 1)) / PAIRS
                    units.append((lo + (hi - lo) * fa, mk_normA(p, state, pend)))
                    # PE half deferred ~3 units into the next pair so the DVE
                    # recip/mul chain hides behind that pair's score matmuls
                    fb = (p + (kk_hi + 4.0) / (kk_hi + 1)) / PAIRS
                    if p < PAIRS - 1:
                        units.append((lo + (hi - lo) * fb, mk_normB(p, state)))
                    else:
                        last_normB = mk_normB(p, state)
                return units, last_normB

            # ---- projection (wide stage, 1 DMA per 4 groups) ----
            outr = out_d.rearrange("(o p) t -> p o t", p=128)

            def proj_groups(j, borrow=False):
                """Emission order of the 8 groups is free: the wide stage
                tile per 4-group slot is created on first use, its DMA issued
                when the slot's last copy lands."""
                qsl0 = 512 * j
                wide = {}
                left = {0: 4, 1: 4}

                def g(o):
                    if borrow and o % 3 == 1:
                        pp = st_region()[:, 0, :]
                    elif borrow and o % 3 == 2:
                        pp = acc_region().rearrange("p a b -> p (a b)")
                    else:
                        pp = mm_region()
                    for cpt in range(4):
                        nc.tensor.matmul(pp, wp2[:, cpt, 128 * o:128 * o + 128],
                                         yT[cpt][:, bass.ds(qsl0, 512)],
                                         start=(cpt == 0), stop=(cpt == 3))
                    sl = o // 4
                    if sl not in wide:
                        wide[sl] = stage_p.tile([128, 4, 512], DT, tag="stg",
                                                name=uname("stg"))
                    stg = wide[sl]
                    nc.vector.tensor_copy(stg[:, o % 4, :], pp)
                    left[sl] -= 1
                    if left[sl] == 0:
                        nc.sync.dma_start(
                            outr[:, 4 * sl:4 * sl + 4, qsl0:qsl0 + 512], stg)
                return [lambda o=o: g(o) for o in range(8)]

            def weave(items):
                for _, fn in sorted(items, key=lambda t: t[0]):
                    fn()

            # ================= window 0 =================
            # kc-major qk across all 8 head-blocks: each arriving weight chunk
            # unlocks an 8-matmul burst, so PE streams at the DMA issue rate.
            # Borrowed PSUM regions are safe here: the prefix fully emits
            # (writes + bias-add reads) before any attention allocation.
            st0, st1 = st_region(), st_region()
            a0 = acc_region().rearrange("p a b -> p (a b)")
            a1 = acc_region().rearrange("p a b -> p (a b)")
            regions = [mm_region(), mm_region(), st0[:, 0, :], st0[:, 1, :],
                       st1[:, 0, :], st1[:, 1, :], a0, a1]
            for kc in range(CT):
                for m in range(8):
                    nc.tensor.matmul(regions[m], wqkt(kc)[:, 128 * m:128 * m + 128],
                                     xTt(kc)[:, 0:512],
                                     start=(kc == 0), stop=(kc == CT - 1),
                                     skip_group_check=True)
            for m in range(8):
                nc.vector.tensor_scalar_add(qkT[m][:, 0:512], regions[m],
                                            bqk[:, m:m + 1])

            # ---- block-3 projection split: cpt 0..2 in window 3 (pair-3
            # filler), the pair-3 term + staged-partial add in the drain ----
            def proj3_partial_groups():
                qsl0 = 512 * (NT - 1)

                def g(o):
                    pp = mm_region()
                    for cpt in range(3):
                        nc.tensor.matmul(pp, wp2[:, cpt, 128 * o:128 * o + 128],
                                         yT[cpt][:, bass.ds(qsl0, 512)],
                                         start=(cpt == 0), stop=(cpt == 2))
                    nc.vector.tensor_copy(prt[o], pp)
                return [lambda o=o: g(o) for o in range(8)]

            def proj3_finish():
                # Phase A: staged-partial adds (start=True) for the first six
                # groups run while the carry's DVE norm chain still computes
                # yT3 -- they only need prt. Phase B adds the pair-3 term,
                # stages and ships. Groups 6/7 wait on borrow-ring slots.
                qsl0 = 512 * (NT - 1)
                wide = {}
                pps = {}
                pair_left = {0: 2, 1: 2, 2: 2, 3: 2}

                def region(o):
                    if o % 3 == 0:
                        return st_region()[:, 0, :]
                    if o % 3 == 1:
                        return acc_region().rearrange("p a b -> p (a b)")
                    return mm_region()

                def phase_a(o):
                    pps[o] = region(o)
                    nc.tensor.matmul(pps[o], identb, prt[o],
                                     start=True, stop=False)

                def phase_b(o):
                    pp = pps[o]
                    nc.tensor.matmul(pp, wp2[:, 3, 128 * o:128 * o + 128],
                                     yT[3][:, bass.ds(qsl0, 512)],
                                     start=False, stop=True)
                    sl = o // 4
                    if sl not in wide:
                        wide[sl] = stage_p.tile([128, 4, 512], DT, tag="stgd",
                                                name=uname("stgd"))
                    stg = wide[sl]
                    if o % 2 == 0:
                        nc.vector.tensor_copy(stg[:, o % 4, :], pp)
                    else:
                        nc.scalar.activation(stg[:, o % 4, :], pp,
                                             mybir.ActivationFunctionType.Copy)
                    pq = o // 2
                    pair_left[pq] -= 1
                    if pair_left[pq] == 0:
                        o0, s0 = 2 * pq, (2 * pq) % 4
                        nc.sync.dma_start(outr[:, o0:o0 + 2, qsl0:qsl0 + 512],
                                          stg[:, s0:s0 + 2, :])

                # phase A takes only st/acc borrow slots: the carry's
                # transposes still need a free mm-ring slot
                def a_part():
                    for o in (0, 1, 3, 4):
                        phase_a(o)

                def b_part():
                    for o in (0, 1, 3, 4):
                        phase_b(o)
                    for o in (2, 5, 6, 7):
                        phase_a(o)
                        phase_b(o)

                return a_part, b_part

            # Pure-PE filler (v/proj/next-window q,k groups) sits INSIDE the
            # first units of each pair: the score pipeline outruns exp there
            # (no AV backlog yet), so scores(kk2) stalls on the st ring slot
            # whose exp(kk0) is still running unless filler covers it.
            items = []
            items.append((0.00, v_group(0)))
            items.append((0.06, v_group(1)))
            items.append((0.12, v_group(2)))
            items.append((0.18, v_group(3)))
            items.append((0.23, lambda: qk_group(1, 0)))
            items.append((0.31, lambda: qk_group(1, 1)))
            items.append((0.48, lambda: qk_group(1, 2)))
            items.append((0.56, lambda: qk_group(1, 3)))
            items.append((0.73, lambda: qk_group(1, 4)))
            items.append((0.81, lambda: qk_group(1, 5)))
            au, carry = attn_units(0, lo=0.02, hi=1.0)
            items += au
            weave(items)

            # ================= windows 1..3 =================
            # dependency-free k-groups are the in-pair fillers (proj groups
            # can stall on the mm ring); v-groups pad pair 0's pipeline fill
            for n in range(1, NT):
                kk_hi = 4 * n + 4
                u = 1.0 / ((kk_hi + 1) * 4)
                items = []
                items.append((-0.02, v_group(4 * n + 0)))
                items.append((1.0 * u, v_group(4 * n + 1)))
                items.append((2.2 * u, v_group(4 * n + 2)))
                kbase = 6 if n == 1 else 4  # w0 prefetched k(1,4/5)
                kg = [lambda n=n, p=p: qk_group(n, 4 + p)
                      for p in range(kbase - 4, PAIRS)]
                items.append((3.4 * u, kg[0]))
                items.append((4.6 * u, carry))
                items.append((5.8 * u, v_group(4 * n + 3)))
                pg = proj_groups(n - 1)
                if n == NT - 1:
                    # double pre-fill at the late boundaries where the exp
                    # backlog is deepest
                    items.append((2 / 4 - 0.035, pg[0]))
                    items.append((3 / 4 - 0.035, pg[1]))
                else:
                    items.append((7.0 * u, pg[0]))
                    items.append((8.2 * u, pg[1]))
                for i, pb in enumerate((1, 2, 3)):
                    items.append((pb / 4 - 0.02, pg[2 * pb]))
                    ki = i + 1
                    if ki < len(kg):
                        items.append((pb / 4 + 1.0 * u, kg[ki]))
                        items.append((pb / 4 + 2.6 * u, pg[2 * pb + 1]))
                    else:
                        items.append((pb / 4 + 1.0 * u, pg[2 * pb + 1]))
                hoist = ([lambda n=n, m=m: qk_group(n + 1, m) for m in range(4)]
                         if n < NT - 1 else [])
                for i, g in enumerate(hoist):
                    items.append((0.30 + 0.18 * i, g))
                if n == NT - 1:
                    pp3 = proj3_partial_groups()
                    for i, g in enumerate(pp3[:6]):
                        items.append((0.805 + 0.025 * i, g))
                    # last two partials sit past pair-3's normA so they cover
                    # the drain-entry latency of the carry's DVE chain
                    items.append((0.995, pp3[6]))
                    items.append((0.999, pp3[7]))
                au, carry = attn_units(n)
                items += au
                weave(items)

            # ================= drain =================
            finish_a, finish_b = proj3_finish()
            finish_a()
            carry()
            finish_b()

    nc.compile()
    _cache[dt_name] = nc
    return nc


def make_masks(dt_name=DT_NAME):
    np_dt = _np_dt(dt_name)
    kk = np.arange(128)[:, None]
    qq = np.arange(128)[None, :]
    r = qq - kk
    m = ((r >= 0) & ~((r == 1) & (qq % 2 == 1))).astype(np_dt)
    masks = np.zeros((128, 256), dtype=np_dt)
    masks[:, 0:128] = m
    masks[:, 128:256] = m
    return masks


def prep_inputs(x, w_attn, b_attn, w_proj, dt_name=DT_NAME):
    np_dt = _np_dt(dt_name)
    x = np.asarray(x, dtype=np.float32)
    w_attn = np.asarray(w_attn, dtype=np.float32)
    b_attn = np.asarray(b_attn, dtype=np.float32)
    masks = make_masks(dt_name)
    ident = np.eye(128, dtype=np.float32)
    in_maps = []
    for c in range(G):
        b, g = c // 2, c % 2
        sq = slice(512 * g, 512 * g + 512)
        sk = slice(C + 512 * g, C + 512 * g + 512)
        sv = slice(2 * C + 512 * g, 2 * C + 512 * g + 512)
        wqkT = np.ascontiguousarray(
            np.concatenate([w_attn[sq], w_attn[sk]], axis=0).T.astype(np_dt))
        wvT = np.ascontiguousarray(w_attn[sv].T.astype(np_dt))
        wprojT = np.ascontiguousarray(
            np.asarray(w_proj, np.float32)[:, 512 * g:512 * g + 512].T.astype(np_dt))
        bqk = np.concatenate([b_attn[sq], b_attn[sk]]).reshape(8, 128).T
        cstf = np.ascontiguousarray(
            np.concatenate([bqk, ident], axis=1).astype(np.float32))
        bvr = np.tile(b_attn[sv].reshape(1, 512), (128, 1))
        cstb = np.ascontiguousarray(
            np.concatenate([masks.astype(np.float32), bvr, ident],
                           axis=1).astype(np_dt))
        xT = np.ascontiguousarray(x[b].T.astype(np_dt))
        in_maps.append({
            "xT": xT, "wqkT": wqkT, "wvT": wvT, "wprojT": wprojT,
            "cstf": cstf, "cstb": cstb,
        })
    return in_maps


def unshard(results, b_proj):
    out = np.empty((B, T, C), dtype=np.float32)
    for b in range(B):
        part = (results[2 * b]["out"].astype(np.float32)
                + results[2 * b + 1]["out"].astype(np.float32))
        out[b] = part.T + np.asarray(b_proj, np.float32)[None, :]
    return out


def kernel(x, w_attn, b_attn, w_proj, b_proj):
    nc = build_nc(DT_NAME)
    in_maps = prep_inputs(x, w_attn, b_attn, w_proj, DT_NAME)
    res = run_bass_kernel_spmd(nc, in_maps, list(range(G)))
    return unshard(res.results, b_proj)


if __name__ == "__main__":
    rng = np.random.default_rng(0)
    x = rng.standard_normal((B, T, C), dtype=np.float32)
    w_attn = (rng.standard_normal((3 * C, C), dtype=np.float32) * 0.02)
    b_attn = np.zeros(3 * C, np.float32)
    w_proj = (rng.standard_normal((C, C), dtype=np.float32) * 0.02)
    b_proj = np.zeros(C, np.float32)
    out = kernel(x, w_attn, b_attn, w_proj, b_proj)
    print("out shape:", out.shape, out.dtype)
